# revision 4
# baseline (speedup 1.0000x reference)
"""GQA attention kernel for Trainium2, 8 NeuronCores.

Problem: B=4, T=2048, E=2048, H=16 query heads, KV=4 kv heads, HD=128,
RoPE + causal softmax + output projection. Returns (out, k_cache, v_cache).

Sharding: core i handles batch b = i//2 and kv-head pair gp = i%2
(kv heads {2gp, 2gp+1}, query heads {8gp..8gp+7}).  Each core computes a
partial out[b] = y_heads @ Wo_rows; the host sums the two partials per batch.
No cross-core communication.

On-core dataflow (all matmul operands bf16, fp32 PSUM accumulation):
  - host passes x[b] pre-transposed (xT: [E,T]) so the contraction dim E is
    on partitions for Q/K/V projections.
  - qT/kT produced per head as [HD, T] (transposed) -> RoPE on DVE.
  - v produced in natural layout [T, 2*HD].
  - scores computed transposed: sT[tk,tq] = kT-block.T @ qT-block so the
    PV matmul (lhsT=v natural chunk, rhs=pT) needs no transposes.
  - softmax without max subtraction (logits bounded ~±5 here), causal
    blocks skipped, masking via 0/1 multiply after exp.
  - rowsums via ones-column matmul on PE; reciprocal on DVE; broadcast
    across partitions via rank-1 PE matmul.
  - two head-waves (4 heads each) so qT SBUF slots recycle and attention
    overlaps the second wave's projections.
"""

import numpy as np
import ml_dtypes
from contextlib import ExitStack

import concourse.bass as bass
import concourse.mybir as mybir
import concourse.tile as tile
from concourse import bacc, bass_utils

BF = mybir.dt.bfloat16
F32 = mybir.dt.float32
NPBF = ml_dtypes.bfloat16

B, T, E = 4, 2048, 2048
H, KV, HD = 16, 4, 128
P = 128
NTB = T // 512    # 4 query/key column blocks of 512
NEC = E // P      # 16 contraction chunks of 128
NH = 8            # query heads per core
NKVC = 2          # kv heads per core
SCALE = float(HD) ** -0.5

_CACHE = {}


def _build_kernel():
    nc = bacc.Bacc("TRN2", target_bir_lowering=False, debug=False,
                   enable_asserts=False, num_devices=8)

    xT = nc.dram_tensor("xT", [E, T], BF, kind="ExternalInput").ap()
    wq = nc.dram_tensor("wq", [E, NH * HD], BF, kind="ExternalInput").ap()
    wkv = nc.dram_tensor("wkv", [E, 2 * NKVC * HD], BF, kind="ExternalInput").ap()
    wo = nc.dram_tensor("wo", [NH * HD, E], BF, kind="ExternalInput").ap()
    cosT = nc.dram_tensor("cosT", [P, T], BF, kind="ExternalInput").ap()
    ssT = nc.dram_tensor("ssT", [P, T], BF, kind="ExternalInput").ap()
    maskT = nc.dram_tensor("maskT", [P, 4 * 512], BF, kind="ExternalInput").ap()

    out_p = nc.dram_tensor("out_p", [T, E], F32, kind="ExternalOutput").ap()
    kT_out = nc.dram_tensor("kT_out", [NKVC, P, T], BF, kind="ExternalOutput").ap()
    v_out = nc.dram_tensor("v_out", [T, NKVC * HD], BF, kind="ExternalOutput").ap()

    with tile.TileContext(nc) as tc, ExitStack() as ctx:
        sb = ctx.enter_context(tc.tile_pool(name="sb", bufs=1))
        ps = ctx.enter_context(tc.tile_pool(name="ps", bufs=1, space="PSUM"))

        # --- constants ---
        cos_sb = sb.tile([P, T], BF, tag="cos", name="cos_sb")
        ss_sb = sb.tile([P, T], BF, tag="ss", name="ss_sb")
        mask_sb = sb.tile([P, 4 * 512], BF, tag="mask", name="mask_sb")
        nc.sync.dma_start(cos_sb[:], cosT[:])
        nc.sync.dma_start(ss_sb[:], ssT[:])
        nc.sync.dma_start(mask_sb[:], maskT[:])
        ones_col = sb.tile([P, 1], BF, tag="onesc", name="ones_col")
        nc.vector.memset(ones_col[:], 1.0)
        ones_row = sb.tile([1, P], F32, tag="onesr", name="ones_row")
        nc.vector.memset(ones_row[:], 1.0)

        # --- weight chunks (resident through projections) ---
        wq_c = []
        wkv_c = []
        for ec in range(NEC):
            wqt = sb.tile([P, NH * HD], BF, tag="wq", bufs=NEC, name=f"wq{ec}")
            nc.sync.dma_start(wqt[:], wq[ec * P:(ec + 1) * P, :])
            wq_c.append(wqt)
            wkvt = sb.tile([P, 2 * NKVC * HD], BF, tag="wkv", bufs=NEC,
                           name=f"wkv{ec}")
            nc.sync.dma_start(wkvt[:], wkv[ec * P:(ec + 1) * P, :])
            wkv_c.append(wkvt)

        # --- persistent SBUF tensors ---
        kT_sb = [sb.tile([P, T], BF, tag=f"kT{j}", name=f"kT{j}")
                 for j in range(NKVC)]
        v_sb = sb.tile([P, (T // P) * 2 * HD], BF, tag="v", name="v_sb")
        yT_sb = [sb.tile([P, T], BF, tag=f"yT{h}", name=f"yT{h}")
                 for h in range(NH)]

        def rope_drain(dst_bf16_slice, src_psum, tb):
            """dst = src*cos + swap_halves(src)*signed_sin  (cols tb*512..)."""
            c0, c1 = tb * 512, (tb + 1) * 512
            t1 = sb.tile([P, 512], F32, tag="rope1", bufs=2, name="t1")
            t2 = sb.tile([P, 512], F32, tag="rope2", bufs=2, name="t2")
            nc.vector.tensor_mul(t1[:], src_psum[:], cos_sb[:, c0:c1])
            nc.vector.tensor_mul(t2[0:64, :], src_psum[64:128, :], ss_sb[0:64, c0:c1])
            nc.vector.tensor_mul(t2[64:128, :], src_psum[0:64, :], ss_sb[64:128, c0:c1])
            nc.vector.tensor_add(dst_bf16_slice, t1[:], t2[:])

        for w in range(2):          # head wave: heads 4w..4w+3, kv head j=w
            # ---- projections for this wave ----
            qT_sb = [sb.tile([P, T], BF, tag="qT", bufs=4, name=f"qT_w{w}h{hh}")
                     for hh in range(4)]
            for tb in range(NTB):
                c0, c1 = tb * 512, (tb + 1) * 512
                xts = []
                for ec in range(NEC):
                    xt = sb.tile([P, 512], BF, tag="xt", bufs=24, name="xt")
                    nc.sync.dma_start(xt[:], xT[ec * P:(ec + 1) * P, c0:c1])
                    xts.append(xt)
                # K projection for kv head j=w
                kp = ps.tile([P, 512], F32, tag="mm", bufs=5, name="kp")
                for ec in range(NEC):
                    nc.tensor.matmul(kp[:], wkv_c[ec][:, w * HD:(w + 1) * HD],
                                     xts[ec][:], start=(ec == 0), stop=(ec == NEC - 1))
                rope_drain(kT_sb[w][:, c0:c1], kp, tb)
                # V projection (both kv heads, natural layout) in wave 0 only
                if w == 0:
                    for ts4 in range(4):
                        vp = ps.tile([P, 2 * HD], F32, tag="mm", bufs=5, name="vp")
                        for ec in range(NEC):
                            nc.tensor.matmul(vp[:], xts[ec][:, ts4 * P:(ts4 + 1) * P],
                                             wkv_c[ec][:, 2 * HD:4 * HD],
                                             start=(ec == 0), stop=(ec == NEC - 1))
                        chunk = tb * 4 + ts4
                        nc.scalar.activation(v_sb[:, chunk * 256:(chunk + 1) * 256],
                                             vp[:], mybir.ActivationFunctionType.Copy)
                # Q projections for this wave's heads
                for hh in range(4):
                    h = 4 * w + hh
                    qp = ps.tile([P, 512], F32, tag="mm", bufs=5, name="qp")
                    for ec in range(NEC):
                        nc.tensor.matmul(qp[:], wq_c[ec][:, h * HD:(h + 1) * HD],
                                         xts[ec][:], start=(ec == 0),
                                         stop=(ec == NEC - 1))
                    rope_drain(qT_sb[hh][:, c0:c1], qp, tb)

            nc.sync.dma_start(kT_out[w], kT_sb[w][:])
            if w == 0:
                nc.sync.dma_start(
                    v_out.rearrange("(c p) d -> p c d", p=P),
                    v_sb.rearrange("p (c d) -> p c d", d=2 * HD))

            # ---- attention for this wave ----
            for hh in range(4):
                h = 4 * w + hh
                for tqb in range(NTB):
                    q0, q1 = tqb * 512, (tqb + 1) * 512
                    nblk = 4 * tqb + 4
                    yp = ps.tile([P, 512], F32, tag="mm", bufs=5, name="yp")
                    rs = ps.tile([1, 512], F32, tag="rs", bufs=2, name="rs")
                    for tkc in range(nblk):
                        sp = ps.tile([P, 512], F32, tag="mm", bufs=5, name="sp")
                        nc.tensor.matmul(sp[:], kT_sb[w][:, tkc * P:(tkc + 1) * P],
                                         qT_sb[hh][:, q0:q1], start=True, stop=True)
                        pT = sb.tile([P, 512], BF, tag="pT", bufs=6, name="pT")
                        nc.scalar.activation(pT[:], sp[:],
                                             mybir.ActivationFunctionType.Exp,
                                             scale=SCALE)
                        m = tkc - 4 * tqb
                        if m >= 0:  # diagonal block: zero future positions
                            nc.vector.tensor_mul(pT[:], pT[:],
                                                 mask_sb[:, m * 512:(m + 1) * 512])
                        st, last = (tkc == 0), (tkc == nblk - 1)
                        nc.tensor.matmul(
                            yp[:],
                            v_sb[:, tkc * 256 + w * HD:tkc * 256 + (w + 1) * HD],
                            pT[:], start=st, stop=last)
                        nc.tensor.matmul(rs[:], ones_col[:], pT[:],
                                         start=st, stop=last)
                    recip = sb.tile([1, 512], F32, tag="recip", bufs=2, name="recip")
                    nc.vector.reciprocal(recip[:], rs[:])
                    rbc_p = ps.tile([P, 512], F32, tag="mm", bufs=5, name="rbc_p")
                    nc.tensor.matmul(rbc_p[:], ones_row[:], recip[:],
                                     start=True, stop=True)
                    rbc = sb.tile([P, 512], F32, tag="rbc", bufs=2, name="rbc")
                    nc.scalar.activation(rbc[:], rbc_p[:],
                                         mybir.ActivationFunctionType.Copy)
                    nc.vector.tensor_mul(yT_sb[h][:, q0:q1], yp[:], rbc[:])

        # ---------------- output projection ----------------
        for eb in range(4):
            e0, e1 = eb * 512, (eb + 1) * 512
            wos = []
            for h in range(NH):
                wot = sb.tile([P, 512], BF, tag="wo", bufs=NH, name="wot")
                nc.sync.dma_start(wot[:], wo[h * P:(h + 1) * P, e0:e1])
                wos.append(wot)
            for tq in range(T // P):
                op = ps.tile([P, 512], F32, tag="mm", bufs=5, name="op")
                for h in range(NH):
                    nc.tensor.matmul(op[:], yT_sb[h][:, tq * P:(tq + 1) * P],
                                     wos[h][:], start=(h == 0), stop=(h == NH - 1))
                ob = sb.tile([P, 512], F32, tag="ob", bufs=3, name="ob")
                nc.scalar.activation(ob[:], op[:],
                                     mybir.ActivationFunctionType.Copy)
                nc.sync.dma_start(out_p[tq * P:(tq + 1) * P, e0:e1], ob[:])

    nc.compile()
    return nc


def _get_nc():
    if "nc" not in _CACHE:
        _CACHE["nc"] = _build_kernel()
    return _CACHE["nc"]


def _make_mask():
    # mask pattern m (= tkc - 4*tqb): allowed iff c >= r + 128*m
    r = np.arange(P)[:, None]
    c = np.arange(512)[None, :]
    cols = [(c >= r + 128 * m).astype(np.float32) for m in range(4)]
    return np.concatenate(cols, axis=1)


def kernel(x, cos, sin, Wq, Wk, Wv, Wo, trace=False):
    x = np.asarray(x, np.float32)
    cos = np.asarray(cos, np.float32)
    sin = np.asarray(sin, np.float32)
    Wq = np.asarray(Wq, np.float32)
    Wk = np.asarray(Wk, np.float32)
    Wv = np.asarray(Wv, np.float32)
    Wo = np.asarray(Wo, np.float32)

    cosT = np.ascontiguousarray(cos.T).astype(NPBF)          # [HD, T]
    sinT = sin.T
    ssT = np.ascontiguousarray(
        np.concatenate([-sinT[:64], sinT[64:]], axis=0)).astype(NPBF)
    maskT = _make_mask().astype(NPBF)

    in_maps = []
    for i in range(8):
        b, gp = i // 2, i % 2
        in_maps.append({
            "xT": np.ascontiguousarray(x[b].T).astype(NPBF),
            "wq": np.ascontiguousarray(
                Wq[:, gp * 1024:(gp + 1) * 1024]).astype(NPBF),
            "wkv": np.ascontiguousarray(np.concatenate(
                [Wk[:, gp * 256:(gp + 1) * 256],
                 Wv[:, gp * 256:(gp + 1) * 256]], axis=1)).astype(NPBF),
            "wo": np.ascontiguousarray(
                Wo[gp * 1024:(gp + 1) * 1024, :]).astype(NPBF),
            "cosT": cosT, "ssT": ssT, "maskT": maskT,
        })

    nc = _get_nc()
    res = bass_utils.run_bass_kernel_spmd(nc, in_maps, core_ids=list(range(8)),
                                          trace=trace)
    _CACHE["last_results"] = res

    out = np.empty((B, T, E), np.float32)
    k = np.empty((B, KV, T, HD), np.float32)
    v = np.empty((B, KV, T, HD), np.float32)
    for i in range(8):
        b, gp = i // 2, i % 2
        r = res.results[i]
        if gp == 0:
            out[b] = r["out_p"]
        else:
            out[b] += r["out_p"]
        for j in range(NKVC):
            k[b, 2 * gp + j] = r["kT_out"][j].T.astype(np.float32)
            v[b, 2 * gp + j] = r["v_out"][:, j * HD:(j + 1) * HD].astype(np.float32)
    return out, k, v
